# revision 10
# baseline (speedup 1.0000x reference)
"""DiffGraphAttentionLayer Trainium2 kernel (8 NeuronCores, SPMD).

Shapes: B=4, N=2048, IN_F=256, OUT_F=128, D=64, NUM_REL=6.
  Wh = h @ W
  e_s[b,i,j] = L_s[b,i] + R_s[b,j] + T_s[adj[b,i,j]]      (s in {pos,neg})
  attn_s = softmax_j( where(adj>0, lrelu(e_s), -9e15) )
  attention = attn_pos - lambda_full*attn_neg              -> OUTPUT 2 [B,N,N]
  out = gelu(LN(attention @ Wh) * (1-l0))                  -> OUTPUT 1 [B,N,128]

Sharding: core c <- (batch b = c//2, row half = c%2): each core owns 1024
attention rows (full j range). No collectives needed.

Device pipeline per i-tile (128 rows x 2048 cols):
  z    = float(adj) - 3                                   (tensor_scalar)
  q_s  = quintic(z) via 1 TS + 4 fused scalar_tensor_tensor passes; the
         quintic interpolates alpha_s*T_s[a] at z=-2..2 and a large negative
         value at z=-3 (adj==0), which implements the softmax mask.
  e~   = (q_s + (alpha_s*L_s[i]+r0_s)) + alpha_s*R_s[j]   (stt; R broadcast
         tile static per core) ; this equals alpha_s * e_s + const(i)
  v    = max(e~, 0.2 e~) = alpha_s*lrelu(e_s) + const     (stt)
  p_s  = exp(v/alpha_s), row-sums via ACT accum_out
  attn = p_pos*(1/s_pos) + p_neg*(-lambda/s_neg)          (TS + stt)
  attn^T via PE transposes; hp[i,:] = sum_j attn^T_jb @ Wh_jb (PE, bf16)
  epilogue: LN (bn_stats) + gelu (ACT) on [128,128] tiles.

Host does only O(N*IN_F) prep: the 6-entry relation tables, lambda, the
L/R projections (h @ (W @ a) for 4 tiny vectors) and the quintic fits.
"""
import os
import sys
sys.path.insert(0, '/opt/trn_rl_repo')

import numpy as np
from contextlib import ExitStack

import concourse.bass as bass
import concourse.tile as tile
from concourse import mybir
from concourse.bass_utils import run_bass_kernel_spmd
from concourse.masks import make_identity
from concourse.vector_clock import ScopedClock, VectorClock
from concourse.tile_sem_assignment import N_PROCS

# ---------------------------------------------------------------- constants
B, N, IN_F, OUT_F = 4, 2048, 256, 128
D = OUT_F // 2
ALPHA_LRELU = 0.2
LAMBDA_INIT = 0.8 - 0.6 * float(np.exp(-0.3 * 1))
EPS = 1e-5
ROWS = N // 2          # rows per core (1024)
NT = ROWS // 128       # i-tiles per core (8)
JT = N // 128          # j-blocks (16)
MASK_C = 250.0         # masked e value; exp(0.2*(-250+|L+R|)) ~ 1e-20 -> 0
F32 = mybir.dt.float32
BF16 = mybir.dt.bfloat16
I32 = mybir.dt.int32
AX = mybir.AluOpType
AF = mybir.ActivationFunctionType

_cache = {}


def _patch_tile_drain():
    """This container's walrus rejects >N sem waits on one Drain ("Too many
    sync wait commands"). Spread the kernel-tail global-clock waits across
    single-wait NOPs on the sync engine."""
    if getattr(tile.TileContext, "_dga_drain_patched", False):
        return

    def _drain_and_barrier_split(self, tick_clock, wait_clock):
        nc = self.nc
        g = tick_clock.global_clock
        for p in range(N_PROCS):
            if g[p]:
                vec = [0] * N_PROCS
                vec[p] = g[p]
                nop = nc.sync.nop(hint="drain_split", nofuse=True)
                wait_clock.add_sem_waits(nop.ins, ScopedClock({None: VectorClock(vec)}))
        nc.sync.drain()
        nc.all_engine_barrier()
        assert self.sems is not None
        popped = nc._tile_sem_poison_stack.pop()
        assert popped is self._sem_poison
        nc.clear_and_free_semaphores(list(self.sems.allocated().values()))
        nc.all_engine_barrier()

    tile.TileContext._drain_and_barrier = _drain_and_barrier_split
    tile.TileContext._dga_drain_patched = True



def _patch_bir_wait_legalize():
    """This walrus build caps sem-waits per instruction (errors with "Too
    many sync wait commands"). Legalize the BIR before compile: move excess
    on_wait entries onto NoOp instructions inserted just before, same engine."""
    import orjson
    import concourse.bass_utils as bu
    import concourse.bass2jax as b2j
    if getattr(bu, "_dga_wait_legalized", False):
        return
    inner = bu.compile_bir_kernel

    def _legalize(bir_json: bytes) -> bytes:
        d = orjson.loads(bir_json)
        cnt = [0]

        def fix_list(lst):
            out = []
            for inst in lst:
                si = inst.get("sync_info")
                waits = si.get("on_wait") if si else None
                if waits and len(waits) > 1:
                    for w in waits[:-1]:
                        cnt[0] += 1
                        out.append({
                            "debug": inst.get("debug", 0),
                            "engine": inst["engine"],
                            "ins": [], "outs": [],
                            "name": inst["name"] + "_xw" + str(cnt[0]),
                            "opcode": "NoOp",
                            "sync_info": {"on_update": [], "on_wait": [w]},
                            "text_hint": "wait_split",
                        })
                    si["on_wait"] = [waits[-1]]
                out.append(inst)
            return out

        def walk(o):
            if isinstance(o, dict):
                if isinstance(o.get("instructions"), list):
                    o["instructions"] = fix_list(o["instructions"])
                for v in o.values():
                    walk(v)
            elif isinstance(o, list):
                for v in o:
                    walk(v)

        walk(d)
        return orjson.dumps(d)

    def wrapped(bir_json, tmpdir, neff_name="file.neff"):
        return inner(_legalize(bir_json), tmpdir, neff_name=neff_name)

    bu.compile_bir_kernel = wrapped
    b2j.compile_bir_kernel = wrapped
    bu._dga_wait_legalized = True


def _fit_chain(T_vals, alpha):
    """Quintic p(z) = k1 z^5 + ... + k5 z + r0 interpolating alpha*T_vals[a]
    at z=a-3 (a=1..5) and -alpha*MASK_C at z=-3. Chain form:
    a1 = k1*z; a_{m+1} = (a_m + k_{m+1})*z; p = a5 + r0."""
    zs = np.arange(6, dtype=np.float64) - 3.0
    ys = np.concatenate([[-MASK_C * alpha], alpha * np.asarray(T_vals, np.float64)])
    V = np.vander(zs, 6, increasing=True)
    c = np.linalg.solve(V, ys)
    return [float(c[5]), float(c[4]), float(c[3]), float(c[2]), float(c[1])], float(c[0])


def _build_nc(consts):
    nc = bass.Bass()
    adj_in = nc.declare_dram_parameter("adj_rows", [ROWS, N], I32, isOutput=False)
    h_in = nc.declare_dram_parameter("h_b", [N, IN_F], F32, isOutput=False)
    w_in = nc.declare_dram_parameter("w_mat", [IN_F, OUT_F], F32, isOutput=False)
    lr_in = nc.declare_dram_parameter("lr_rows", [ROWS, 2], F32, isOutput=False)
    rr_in = nc.declare_dram_parameter("rrow", [2, N], F32, isOutput=False)
    gb_in = nc.declare_dram_parameter("gamma_beta", [1, 2 * OUT_F], F32, isOutput=False)
    attn_out = nc.declare_dram_parameter("attn_rows", [ROWS, N], F32, isOutput=True)
    out_out = nc.declare_dram_parameter("out_rows", [ROWS, OUT_F], F32, isOutput=True)

    kpos, r0pos = consts["kpos"], consts["r0pos"]
    kneg, r0neg = consts["kneg"], consts["r0neg"]
    inv_as = (consts["inv_a_pos"], consts["inv_a_neg"])
    ks_all = (kpos, kneg)
    neg_lambda = consts["neg_lambda"]
    one_minus_l0 = consts["one_minus_l0"]

    with tile.TileContext(nc) as tc, ExitStack() as ctx:
        cp = ctx.enter_context(tc.tile_pool(name="const", bufs=1))
        stage = ctx.enter_context(tc.tile_pool(name="stage", bufs=2))
        ring = ctx.enter_context(tc.tile_pool(name="ring", bufs=2))
        ring3 = ctx.enter_context(tc.tile_pool(name="ring3", bufs=3))
        small = ctx.enter_context(tc.tile_pool(name="small", bufs=4))
        pbig = ctx.enter_context(tc.tile_pool(name="pbig", bufs=1, space="PSUM"))
        psm = ctx.enter_context(tc.tile_pool(name="psm", bufs=2, space="PSUM"))

        ident = cp.tile([128, 128], F32, tag="ident")
        make_identity(nc, ident)

        # ---------------- phase 0 ----------------------------------------
        hT = [ring.tile([128, N], F32, name=f"hT{k}", tag="atT") for k in range(2)]
        w_sb = [cp.tile([128, OUT_F], F32, name=f"w_sb{k}", tag=f"w_sb{k}") for k in range(2)]
        gb_sb = cp.tile([1, 2 * OUT_F], F32, tag="gb_sb")
        rr_sb = [cp.tile([1, N], F32, name=f"rr_sb{s}", tag=f"rr_sb{s}") for s in range(2)]
        nc.sync.dma_start(out=gb_sb[:], in_=gb_in[:])
        for s in range(2):
            nc.sync.dma_start(out=rr_sb[s][:], in_=rr_in[s:s + 1, :])
        for k in range(2):
            nc.sync.dma_start(out=w_sb[k][:], in_=w_in[k * 128:(k + 1) * 128, :])
        lr0 = [cp.tile([128, 2], F32, name=f"lr0_{it}", tag=f"lr0_{it}") for it in range(NT)]
        for it in range(NT):
            nc.sync.dma_start(out=lr0[it][:], in_=lr_in[it * 128:(it + 1) * 128, :])

        for it in range(16):
            h_t = stage.tile([128, IN_F], F32, tag="h_t")
            nc.sync.dma_start(out=h_t[:], in_=h_in[it * 128:(it + 1) * 128, :])
            for k in range(2):
                pt = psm.tile([128, 128], F32, tag="ps")
                nc.tensor.transpose(pt[:], h_t[:, k * 128:(k + 1) * 128], ident)
                nc.scalar.copy(hT[k][:, it * 128:(it + 1) * 128], pt[:])

        whv = [cp.tile([128, OUT_F], F32, name=f"whv{jb}", tag=f"whv{jb}") for jb in range(JT)]
        for jb in range(JT):
            pwh = psm.tile([128, OUT_F], F32, tag="ps")
            for k in range(2):
                nc.tensor.matmul(pwh[:], hT[k][:, jb * 128:(jb + 1) * 128],
                                 w_sb[k][:], start=(k == 0), stop=(k == 1))
            nc.scalar.copy(whv[jb][:], pwh[:])

        ones_col = cp.tile([1, 128], F32, tag="ones_col")
        nc.vector.memset(ones_col[:], 1.0)
        rb = [cp.tile([128, N], F32, name=f"rb{s}", tag=f"rb{s}") for s in range(2)]
        for s in range(2):
            pb = pbig.tile([128, N], F32, tag="pbig")
            for c4 in range(4):
                nc.tensor.matmul(pb[:, c4 * 512:(c4 + 1) * 512], ones_col[:],
                                 rr_sb[s][0:1, c4 * 512:(c4 + 1) * 512],
                                 start=True, stop=True)
            nc.scalar.copy(rb[s][:], pb[:])

        gamma_b = cp.tile([128, OUT_F], F32, tag="gamma_b")
        beta_b = cp.tile([128, OUT_F], F32, tag="beta_b")
        for dst, lo in ((gamma_b, 0), (beta_b, OUT_F)):
            pg = psm.tile([128, OUT_F], F32, tag="ps")
            nc.tensor.matmul(pg[:], ones_col[:], gb_sb[0:1, lo:lo + OUT_F],
                             start=True, stop=True)
            nc.scalar.copy(dst[:], pg[:])

        # ---------------- phase 1 ----------------------------------------
        h_acc = [cp.tile([128, OUT_F], F32, name=f"hacc{it}", tag=f"hacc{it}") for it in range(NT)]
        for it in range(NT):
            adj_t = ring3.tile([128, N], I32, tag="adj")
            nc.sync.dma_start(out=adj_t[:], in_=adj_in[it * 128:(it + 1) * 128, :])
            z_t = ring.tile([128, N], F32, tag="z")
            nc.vector.tensor_scalar(z_t[:], adj_t[:], 3.0, None, AX.subtract)
            p_t = [None, None]
            s_t = small.tile([128, 2], F32, tag="s")
            for s in range(2):
                ks = ks_all[s]
                a_t = ring.tile([128, N], F32, tag=f"a{s}")
                nc.vector.tensor_scalar(a_t[:], z_t[:], ks[0], None, AX.mult)
                for m in range(1, 5):
                    nc.vector.scalar_tensor_tensor(a_t[:], a_t[:], ks[m], z_t[:],
                                                   AX.add, AX.mult)
                nc.vector.scalar_tensor_tensor(a_t[:], a_t[:], lr0[it][:, s:s + 1],
                                               rb[s][:], AX.add, AX.add)
                nc.vector.scalar_tensor_tensor(a_t[:], a_t[:], ALPHA_LRELU, a_t[:],
                                               AX.mult, AX.max)
                p = ring.tile([128, N], F32, tag=f"p{s}")
                nc.scalar.activation(p[:], a_t[:], AF.Exp, bias=0.0,
                                     scale=inv_as[s], accum_out=s_t[:, s:s + 1])
                p_t[s] = p
            c_t = small.tile([128, 2], F32, tag="c")
            nc.vector.reciprocal(c_t[:], s_t[:])
            cn_t = small.tile([128, 1], F32, tag="cn")
            nc.vector.tensor_scalar(cn_t[:], c_t[:, 1:2], neg_lambda, None, AX.mult)
            attn_t = ring.tile([128, N], F32, tag="attn")
            nc.vector.tensor_scalar(attn_t[:], p_t[0][:], c_t[:, 0:1], None, AX.mult)
            nc.vector.scalar_tensor_tensor(attn_t[:], p_t[1][:], cn_t[:, 0:1],
                                           attn_t[:], AX.mult, AX.add)
            nc.sync.dma_start(out=attn_out[it * 128:(it + 1) * 128, :], in_=attn_t[:])
            at_ps = pbig.tile([128, N], F32, tag="pbig")
            for jb in range(JT):
                nc.tensor.transpose(at_ps[:, jb * 128:(jb + 1) * 128],
                                    attn_t[:, jb * 128:(jb + 1) * 128], ident)
            atT = ring.tile([128, N], F32, tag="atT")
            nc.scalar.copy(atT[:], at_ps[:])
            hp_ps = psm.tile([128, OUT_F], F32, tag="ps")
            for jb in range(JT):
                nc.tensor.matmul(hp_ps[:], atT[:, jb * 128:(jb + 1) * 128],
                                 whv[jb][:], start=(jb == 0), stop=(jb == JT - 1))
            nc.scalar.copy(h_acc[it][:], hp_ps[:])

        # ---------------- phase 2: layernorm + gelu -----------------------
        for it in range(NT):
            x = h_acc[it]
            stats = small.tile([128, nc.vector.BN_STATS_DIM], F32, tag="stats")
            mv = small.tile([128, nc.vector.BN_AGGR_DIM], F32, tag="mv")
            nc.vector.bn_stats(out=stats[:], in_=x[:])
            nc.vector.bn_aggr(out=mv[:], in_=stats[:])
            veps = small.tile([128, 1], F32, tag="veps")
            nc.vector.tensor_scalar(veps[:], mv[:, 1:2], EPS, None, AX.add)
            lnv = small.tile([128, 1], F32, tag="lnv")
            nc.scalar.activation(lnv[:], veps[:], AF.Ln, bias=0.0, scale=1.0)
            rstd = small.tile([128, 1], F32, tag="rstd")
            nc.scalar.activation(rstd[:], lnv[:], AF.Exp, bias=0.0, scale=-0.5)
            rstd_c = small.tile([128, 1], F32, tag="rstd_c")
            nc.vector.tensor_scalar(rstd_c[:], rstd[:], one_minus_l0, None, AX.mult)
            xn = small.tile([128, OUT_F], F32, tag="xn")
            nc.vector.tensor_scalar(xn[:], x[:], mv[:, 0:1], rstd_c[:, 0:1],
                                    AX.subtract, AX.mult)
            t1 = small.tile([128, OUT_F], F32, tag="t1")
            nc.vector.tensor_tensor(t1[:], xn[:], gamma_b[:], AX.mult)
            nc.vector.scalar_tensor_tensor(t1[:], beta_b[:], one_minus_l0, t1[:],
                                           AX.mult, AX.add)
            og = small.tile([128, OUT_F], F32, tag="og")
            gelu_fn = AF.Identity if os.environ.get("DGA_NO_GELU") else AF.Gelu
            nc.scalar.activation(og[:], t1[:], gelu_fn, bias=0.0, scale=1.0)
            nc.sync.dma_start(out=out_out[it * 128:(it + 1) * 128, :], in_=og[:])
    return nc


def _prep(inputs):
    h = np.asarray(inputs["h"], np.float32)
    adj = np.asarray(inputs["adj"])
    if adj.dtype != np.int32:
        adj = adj.astype(np.int32)
    W = np.asarray(inputs["W"], np.float32)
    rel_emb = np.asarray(inputs["rel_emb"], np.float64)
    T_pos = rel_emb @ np.asarray(inputs["a_rel_pos"], np.float64)
    T_neg = rel_emb @ np.asarray(inputs["a_rel_neg"], np.float64)
    lam1 = np.exp(np.sum(np.asarray(inputs["lambda_left1"], np.float64)
                         * np.asarray(inputs["lambda_right1"], np.float64)))
    lam2 = np.exp(np.sum(np.asarray(inputs["lambda_left2"], np.float64)
                         * np.asarray(inputs["lambda_right2"], np.float64)))
    lambda_full = float(lam1 - lam2 + LAMBDA_INIT)

    a_pos = 1.0 / max(1e-6, float(np.max(np.abs(T_pos[1:]))))
    a_neg = 1.0 / max(1e-6, float(np.max(np.abs(T_neg[1:]))))
    kpos, r0pos = _fit_chain(T_pos[1:6], a_pos)
    kneg, r0neg = _fit_chain(T_neg[1:6], a_neg)

    # projections (host, tiny): [B, N] each
    wl_p = W[:, :D] @ np.asarray(inputs["a_left_pos"], np.float32)
    wr_p = W[:, :D] @ np.asarray(inputs["a_right_pos"], np.float32)
    wl_n = W[:, D:] @ np.asarray(inputs["a_left_neg"], np.float32)
    wr_n = W[:, D:] @ np.asarray(inputs["a_right_neg"], np.float32)
    L_pos = h @ wl_p; R_pos = h @ wr_p          # [B, N]
    L_neg = h @ wl_n; R_neg = h @ wr_n

    gb = np.concatenate([np.asarray(inputs["ln_gamma"], np.float32),
                         np.asarray(inputs["ln_beta"], np.float32)])[None, :].copy()

    consts = dict(kpos=kpos, r0pos=r0pos, kneg=kneg, r0neg=r0neg,
                  inv_a_pos=1.0 / a_pos, inv_a_neg=1.0 / a_neg,
                  neg_lambda=-lambda_full,
                  one_minus_l0=float(1.0 - LAMBDA_INIT))
    in_maps = []
    for c in range(8):
        b, half = c // 2, c % 2
        sl = slice(half * ROWS, (half + 1) * ROWS)
        lr = np.stack([np.float32(a_pos) * L_pos[b, sl] + np.float32(r0pos),
                       np.float32(a_neg) * L_neg[b, sl] + np.float32(r0neg)],
                      axis=1).astype(np.float32)
        rr = np.stack([np.float32(a_pos) * R_pos[b],
                       np.float32(a_neg) * R_neg[b]], axis=0).astype(np.float32)
        in_maps.append({
            "adj_rows": np.ascontiguousarray(adj[b, sl, :]),
            "h_b": np.ascontiguousarray(h[b]),
            "w_mat": W,
            "lr_rows": lr,
            "rrow": rr,
            "gamma_beta": gb,
        })
    return consts, in_maps


def _get_nc(consts):
    key = tuple(sorted((k, tuple(v) if isinstance(v, list) else v)
                       for k, v in consts.items()))
    if key not in _cache:
        _patch_tile_drain()
        _patch_bir_wait_legalize()
        _cache[key] = _build_nc(consts)
    return _cache[key]


def kernel(**inputs):
    consts, in_maps = _prep(inputs)
    nc = _get_nc(consts)
    res = run_bass_kernel_spmd(nc, in_maps, list(range(8))).results
    out = np.empty((B, N, OUT_F), np.float32)
    attention = np.empty((B, N, N), np.float32)
    for c in range(8):
        b, half = c // 2, c % 2
        sl = slice(half * ROWS, (half + 1) * ROWS)
        attention[b, sl, :] = res[c]["attn_rows"]
        out[b, sl, :] = res[c]["out_rows"]
    return out, attention


# revision 12
# speedup vs baseline: 1.0249x; 1.0249x over previous
"""DiffGraphAttentionLayer Trainium2 kernel (8 NeuronCores, SPMD).

Shapes: B=4, N=2048, IN_F=256, OUT_F=128, D=64, NUM_REL=6.
  Wh = h @ W
  e_s[b,i,j] = L_s[b,i] + R_s[b,j] + T_s[adj[b,i,j]]      (s in {pos,neg})
  attn_s = softmax_j( where(adj>0, lrelu(e_s), -9e15) )
  attention = attn_pos - lambda_full*attn_neg              -> OUTPUT 2 [B,N,N]
  out = gelu(LN(attention @ Wh) * (1-l0))                  -> OUTPUT 1 [B,N,128]

Sharding: core c <- (batch b = c//2, row half = c%2): each core owns 1024
attention rows (full j range). No collectives needed.

Device pipeline per i-tile (128 rows x 2048 cols):
  z    = float(adj) - 3                                   (tensor_scalar)
  q_s  = quintic(z) via 1 TS + 4 fused scalar_tensor_tensor passes; the
         quintic interpolates alpha_s*T_s[a] at z=-2..2 and a large negative
         value at z=-3 (adj==0), which implements the softmax mask.
  e~   = (q_s + (alpha_s*L_s[i]+r0_s)) + alpha_s*R_s[j]   (stt; R broadcast
         tile static per core) ; this equals alpha_s * e_s + const(i)
  v    = max(e~, 0.2 e~) = alpha_s*lrelu(e_s) + const     (stt)
  p_s  = exp(v/alpha_s), row-sums via ACT accum_out
  attn = p_pos*(1/s_pos) + p_neg*(-lambda/s_neg)          (TS + stt)
  attn^T via PE transposes; hp[i,:] = sum_j attn^T_jb @ Wh_jb (PE, bf16)
  epilogue: LN (bn_stats) + gelu (ACT) on [128,128] tiles.

Host does only O(N*IN_F) prep: the 6-entry relation tables, lambda, the
L/R projections (h @ (W @ a) for 4 tiny vectors) and the quintic fits.
"""
import os
import sys
sys.path.insert(0, '/opt/trn_rl_repo')

import numpy as np
from contextlib import ExitStack

import concourse.bass as bass
import concourse.tile as tile
from concourse import mybir
from concourse.bass_utils import run_bass_kernel_spmd
from concourse.masks import make_identity
from concourse.vector_clock import ScopedClock, VectorClock
from concourse.tile_sem_assignment import N_PROCS

# ---------------------------------------------------------------- constants
B, N, IN_F, OUT_F = 4, 2048, 256, 128
D = OUT_F // 2
ALPHA_LRELU = 0.2
LAMBDA_INIT = 0.8 - 0.6 * float(np.exp(-0.3 * 1))
EPS = 1e-5
ROWS = N // 2          # rows per core (1024)
NT = ROWS // 128       # i-tiles per core (8)
JT = N // 128          # j-blocks (16)
MASK_C = 250.0         # masked e value; exp(0.2*(-250+|L+R|)) ~ 1e-20 -> 0
F32 = mybir.dt.float32
BF16 = mybir.dt.bfloat16
I32 = mybir.dt.int32
AX = mybir.AluOpType
AF = mybir.ActivationFunctionType

_cache = {}


def _patch_tile_drain():
    """This container's walrus rejects >N sem waits on one Drain ("Too many
    sync wait commands"). Spread the kernel-tail global-clock waits across
    single-wait NOPs on the sync engine."""
    if getattr(tile.TileContext, "_dga_drain_patched", False):
        return

    def _drain_and_barrier_split(self, tick_clock, wait_clock):
        nc = self.nc
        g = tick_clock.global_clock
        for p in range(N_PROCS):
            if g[p]:
                vec = [0] * N_PROCS
                vec[p] = g[p]
                nop = nc.sync.nop(hint="drain_split", nofuse=True)
                wait_clock.add_sem_waits(nop.ins, ScopedClock({None: VectorClock(vec)}))
        nc.sync.drain()
        nc.all_engine_barrier()
        assert self.sems is not None
        popped = nc._tile_sem_poison_stack.pop()
        assert popped is self._sem_poison
        nc.clear_and_free_semaphores(list(self.sems.allocated().values()))
        nc.all_engine_barrier()

    tile.TileContext._drain_and_barrier = _drain_and_barrier_split
    tile.TileContext._dga_drain_patched = True



def _patch_bir_wait_legalize():
    """This walrus build caps sem-waits per instruction (errors with "Too
    many sync wait commands"). Legalize the BIR before compile: move excess
    on_wait entries onto NoOp instructions inserted just before, same engine."""
    import orjson
    import concourse.bass_utils as bu
    import concourse.bass2jax as b2j
    if getattr(bu, "_dga_wait_legalized", False):
        return
    inner = bu.compile_bir_kernel

    def _legalize(bir_json: bytes) -> bytes:
        d = orjson.loads(bir_json)
        cnt = [0]

        def fix_list(lst):
            out = []
            for inst in lst:
                si = inst.get("sync_info")
                waits = si.get("on_wait") if si else None
                if waits and len(waits) > 1:
                    for w in waits[:-1]:
                        cnt[0] += 1
                        out.append({
                            "debug": inst.get("debug", 0),
                            "engine": inst["engine"],
                            "ins": [], "outs": [],
                            "name": inst["name"] + "_xw" + str(cnt[0]),
                            "opcode": "NoOp",
                            "sync_info": {"on_update": [], "on_wait": [w]},
                            "text_hint": "wait_split",
                        })
                    si["on_wait"] = [waits[-1]]
                out.append(inst)
            return out

        def walk(o):
            if isinstance(o, dict):
                if isinstance(o.get("instructions"), list):
                    o["instructions"] = fix_list(o["instructions"])
                for v in o.values():
                    walk(v)
            elif isinstance(o, list):
                for v in o:
                    walk(v)

        walk(d)
        return orjson.dumps(d)

    def wrapped(bir_json, tmpdir, neff_name="file.neff"):
        return inner(_legalize(bir_json), tmpdir, neff_name=neff_name)

    bu.compile_bir_kernel = wrapped
    b2j.compile_bir_kernel = wrapped
    bu._dga_wait_legalized = True


def _fit_chain(T_vals, alpha):
    """Quintic p(z) = k1 z^5 + ... + k5 z + r0 interpolating alpha*T_vals[a]
    at z=a-3 (a=1..5) and -alpha*MASK_C at z=-3. Chain form:
    a1 = k1*z; a_{m+1} = (a_m + k_{m+1})*z; p = a5 + r0."""
    zs = np.arange(6, dtype=np.float64) - 3.0
    ys = np.concatenate([[-MASK_C * alpha], alpha * np.asarray(T_vals, np.float64)])
    V = np.vander(zs, 6, increasing=True)
    c = np.linalg.solve(V, ys)
    return [float(c[5]), float(c[4]), float(c[3]), float(c[2]), float(c[1])], float(c[0])


def _build_nc(consts):
    nc = bass.Bass()
    adj_in = nc.declare_dram_parameter("adj_rows", [ROWS, N], I32, isOutput=False)
    h_in = nc.declare_dram_parameter("h_b", [N, IN_F], F32, isOutput=False)
    w_in = nc.declare_dram_parameter("w_mat", [IN_F, OUT_F], F32, isOutput=False)
    lr_in = nc.declare_dram_parameter("lr_rows", [ROWS, 2], F32, isOutput=False)
    rr_in = nc.declare_dram_parameter("rrow", [2, N], F32, isOutput=False)
    gb_in = nc.declare_dram_parameter("gamma_beta", [1, 2 * OUT_F], F32, isOutput=False)
    attn_out = nc.declare_dram_parameter("attn_rows", [ROWS, N], F32, isOutput=True)
    out_out = nc.declare_dram_parameter("out_rows", [ROWS, OUT_F], F32, isOutput=True)

    kpos, r0pos = consts["kpos"], consts["r0pos"]
    kneg, r0neg = consts["kneg"], consts["r0neg"]
    inv_as = (consts["inv_a_pos"], consts["inv_a_neg"])
    ks_all = (kpos, kneg)
    neg_lambda = consts["neg_lambda"]
    one_minus_l0 = consts["one_minus_l0"]

    with tile.TileContext(nc) as tc, ExitStack() as ctx:
        cp = ctx.enter_context(tc.tile_pool(name="const", bufs=1))
        stage = ctx.enter_context(tc.tile_pool(name="stage", bufs=2))
        ring = ctx.enter_context(tc.tile_pool(name="ring", bufs=2))
        ring3 = ctx.enter_context(tc.tile_pool(name="ring3", bufs=3))
        small = ctx.enter_context(tc.tile_pool(name="small", bufs=4))
        pbig = ctx.enter_context(tc.tile_pool(name="pbig", bufs=1, space="PSUM"))
        psm = ctx.enter_context(tc.tile_pool(name="psm", bufs=2, space="PSUM"))

        ident = cp.tile([128, 128], F32, tag="ident")
        make_identity(nc, ident)

        # ---------------- phase 0 ----------------------------------------
        hT = [ring.tile([128, N], F32, name=f"hT{k}", tag="atT") for k in range(2)]
        w_sb = [cp.tile([128, OUT_F], F32, name=f"w_sb{k}", tag=f"w_sb{k}") for k in range(2)]
        gb_sb = cp.tile([1, 2 * OUT_F], F32, tag="gb_sb")
        rr_sb = [cp.tile([1, N], F32, name=f"rr_sb{s}", tag=f"rr_sb{s}") for s in range(2)]
        nc.sync.dma_start(out=gb_sb[:], in_=gb_in[:])
        for s in range(2):
            nc.sync.dma_start(out=rr_sb[s][:], in_=rr_in[s:s + 1, :])
        for k in range(2):
            nc.sync.dma_start(out=w_sb[k][:], in_=w_in[k * 128:(k + 1) * 128, :])
        lr0 = [cp.tile([128, 2], F32, name=f"lr0_{it}", tag=f"lr0_{it}") for it in range(NT)]
        for it in range(NT):
            nc.sync.dma_start(out=lr0[it][:], in_=lr_in[it * 128:(it + 1) * 128, :])

        for it in range(16):
            h_t = stage.tile([128, IN_F], F32, tag="h_t")
            nc.sync.dma_start(out=h_t[:], in_=h_in[it * 128:(it + 1) * 128, :])
            for k in range(2):
                pt = psm.tile([128, 128], F32, tag="ps")
                nc.tensor.transpose(pt[:], h_t[:, k * 128:(k + 1) * 128], ident)
                nc.scalar.copy(hT[k][:, it * 128:(it + 1) * 128], pt[:])

        whv = [cp.tile([128, OUT_F], F32, name=f"whv{jb}", tag=f"whv{jb}") for jb in range(JT)]
        for jb in range(JT):
            pwh = psm.tile([128, OUT_F], F32, tag="ps")
            for k in range(2):
                nc.tensor.matmul(pwh[:], hT[k][:, jb * 128:(jb + 1) * 128],
                                 w_sb[k][:], start=(k == 0), stop=(k == 1))
            nc.scalar.copy(whv[jb][:], pwh[:])

        ones_col = cp.tile([1, 128], F32, tag="ones_col")
        nc.vector.memset(ones_col[:], 1.0)
        rb = [cp.tile([128, N], F32, name=f"rb{s}", tag=f"rb{s}") for s in range(2)]
        for s in range(2):
            pb = pbig.tile([128, N], F32, tag="pbig")
            for c4 in range(4):
                nc.tensor.matmul(pb[:, c4 * 512:(c4 + 1) * 512], ones_col[:],
                                 rr_sb[s][0:1, c4 * 512:(c4 + 1) * 512],
                                 start=True, stop=True)
            nc.scalar.copy(rb[s][:], pb[:])

        gamma_b = cp.tile([128, OUT_F], F32, tag="gamma_b")
        beta_b = cp.tile([128, OUT_F], F32, tag="beta_b")
        for dst, lo in ((gamma_b, 0), (beta_b, OUT_F)):
            pg = psm.tile([128, OUT_F], F32, tag="ps")
            nc.tensor.matmul(pg[:], ones_col[:], gb_sb[0:1, lo:lo + OUT_F],
                             start=True, stop=True)
            nc.scalar.copy(dst[:], pg[:])

        # ---------------- phase 1 ----------------------------------------
        h_acc = [cp.tile([128, OUT_F], F32, name=f"hacc{it}", tag=f"hacc{it}") for it in range(NT)]
        for it in range(NT):
            adj_t = ring3.tile([128, N], I32, tag="adj")
            nc.sync.dma_start(out=adj_t[:], in_=adj_in[it * 128:(it + 1) * 128, :])
            z_t = ring.tile([128, N], F32, tag="z")
            nc.vector.tensor_scalar(z_t[:], adj_t[:], 3.0, None, AX.subtract)
            p_t = [None, None]
            s_t = small.tile([128, 2], F32, tag="s")
            for s in range(2):
                ks = ks_all[s]
                a_t = ring.tile([128, N], F32, tag=f"a{s}")
                nc.vector.tensor_scalar(a_t[:], z_t[:], ks[0], None, AX.mult)
                for m in range(1, 5):
                    nc.vector.scalar_tensor_tensor(a_t[:], a_t[:], ks[m], z_t[:],
                                                   AX.add, AX.mult)
                nc.vector.scalar_tensor_tensor(a_t[:], a_t[:], lr0[it][:, s:s + 1],
                                               rb[s][:], AX.add, AX.add)
                nc.vector.scalar_tensor_tensor(a_t[:], a_t[:], ALPHA_LRELU, a_t[:],
                                               AX.mult, AX.max)
                p = ring.tile([128, N], F32, tag=f"p{s}")
                nc.scalar.activation(p[:], a_t[:], AF.Exp, bias=0.0,
                                     scale=inv_as[s], accum_out=s_t[:, s:s + 1])
                p_t[s] = p
            c_t = small.tile([128, 2], F32, tag="c")
            nc.vector.reciprocal(c_t[:], s_t[:])
            cn_t = small.tile([128, 1], F32, tag="cn")
            nc.vector.tensor_scalar(cn_t[:], c_t[:, 1:2], neg_lambda, None, AX.mult)
            attn_t = ring.tile([128, N], F32, tag="attn")
            nc.scalar.activation(attn_t[:], p_t[0][:], AF.Copy, bias=0.0,
                                 scale=c_t[:, 0:1])
            nc.vector.scalar_tensor_tensor(attn_t[:], p_t[1][:], cn_t[:, 0:1],
                                           attn_t[:], AX.mult, AX.add)
            nc.sync.dma_start(out=attn_out[it * 128:(it + 1) * 128, :], in_=attn_t[:])
            at_ps = pbig.tile([128, N], F32, tag="pbig")
            for jb in range(JT):
                nc.tensor.transpose(at_ps[:, jb * 128:(jb + 1) * 128],
                                    attn_t[:, jb * 128:(jb + 1) * 128], ident)
            atT = ring.tile([128, N], F32, tag="atT")
            nc.scalar.copy(atT[:], at_ps[:])
            hp_ps = psm.tile([128, OUT_F], F32, tag="ps")
            for jb in range(JT):
                nc.tensor.matmul(hp_ps[:], atT[:, jb * 128:(jb + 1) * 128],
                                 whv[jb][:], start=(jb == 0), stop=(jb == JT - 1))
            nc.scalar.copy(h_acc[it][:], hp_ps[:])

        # ---------------- phase 2: layernorm + gelu -----------------------
        for it in range(NT):
            x = h_acc[it]
            stats = small.tile([128, nc.vector.BN_STATS_DIM], F32, tag="stats")
            mv = small.tile([128, nc.vector.BN_AGGR_DIM], F32, tag="mv")
            nc.vector.bn_stats(out=stats[:], in_=x[:])
            nc.vector.bn_aggr(out=mv[:], in_=stats[:])
            veps = small.tile([128, 1], F32, tag="veps")
            nc.vector.tensor_scalar(veps[:], mv[:, 1:2], EPS, None, AX.add)
            lnv = small.tile([128, 1], F32, tag="lnv")
            nc.scalar.activation(lnv[:], veps[:], AF.Ln, bias=0.0, scale=1.0)
            rstd = small.tile([128, 1], F32, tag="rstd")
            nc.scalar.activation(rstd[:], lnv[:], AF.Exp, bias=0.0, scale=-0.5)
            rstd_c = small.tile([128, 1], F32, tag="rstd_c")
            nc.vector.tensor_scalar(rstd_c[:], rstd[:], one_minus_l0, None, AX.mult)
            xn = small.tile([128, OUT_F], F32, tag="xn")
            nc.vector.tensor_scalar(xn[:], x[:], mv[:, 0:1], rstd_c[:, 0:1],
                                    AX.subtract, AX.mult)
            t1 = small.tile([128, OUT_F], F32, tag="t1")
            nc.vector.tensor_tensor(t1[:], xn[:], gamma_b[:], AX.mult)
            nc.vector.scalar_tensor_tensor(t1[:], beta_b[:], one_minus_l0, t1[:],
                                           AX.mult, AX.add)
            og = small.tile([128, OUT_F], F32, tag="og")
            gelu_fn = AF.Identity if os.environ.get("DGA_NO_GELU") else AF.Gelu
            nc.scalar.activation(og[:], t1[:], gelu_fn, bias=0.0, scale=1.0)
            nc.sync.dma_start(out=out_out[it * 128:(it + 1) * 128, :], in_=og[:])
    return nc


def _prep(inputs):
    h = np.asarray(inputs["h"], np.float32)
    adj = np.asarray(inputs["adj"])
    if adj.dtype != np.int32:
        adj = adj.astype(np.int32)
    W = np.asarray(inputs["W"], np.float32)
    rel_emb = np.asarray(inputs["rel_emb"], np.float64)
    T_pos = rel_emb @ np.asarray(inputs["a_rel_pos"], np.float64)
    T_neg = rel_emb @ np.asarray(inputs["a_rel_neg"], np.float64)
    lam1 = np.exp(np.sum(np.asarray(inputs["lambda_left1"], np.float64)
                         * np.asarray(inputs["lambda_right1"], np.float64)))
    lam2 = np.exp(np.sum(np.asarray(inputs["lambda_left2"], np.float64)
                         * np.asarray(inputs["lambda_right2"], np.float64)))
    lambda_full = float(lam1 - lam2 + LAMBDA_INIT)

    a_pos = 1.0 / max(1e-6, float(np.max(np.abs(T_pos[1:]))))
    a_neg = 1.0 / max(1e-6, float(np.max(np.abs(T_neg[1:]))))
    kpos, r0pos = _fit_chain(T_pos[1:6], a_pos)
    kneg, r0neg = _fit_chain(T_neg[1:6], a_neg)

    # projections (host, tiny): [B, N] each
    wl_p = W[:, :D] @ np.asarray(inputs["a_left_pos"], np.float32)
    wr_p = W[:, :D] @ np.asarray(inputs["a_right_pos"], np.float32)
    wl_n = W[:, D:] @ np.asarray(inputs["a_left_neg"], np.float32)
    wr_n = W[:, D:] @ np.asarray(inputs["a_right_neg"], np.float32)
    L_pos = h @ wl_p; R_pos = h @ wr_p          # [B, N]
    L_neg = h @ wl_n; R_neg = h @ wr_n

    gb = np.concatenate([np.asarray(inputs["ln_gamma"], np.float32),
                         np.asarray(inputs["ln_beta"], np.float32)])[None, :].copy()

    consts = dict(kpos=kpos, r0pos=r0pos, kneg=kneg, r0neg=r0neg,
                  inv_a_pos=1.0 / a_pos, inv_a_neg=1.0 / a_neg,
                  neg_lambda=-lambda_full,
                  one_minus_l0=float(1.0 - LAMBDA_INIT))
    in_maps = []
    for c in range(8):
        b, half = c // 2, c % 2
        sl = slice(half * ROWS, (half + 1) * ROWS)
        lr = np.stack([np.float32(a_pos) * L_pos[b, sl] + np.float32(r0pos),
                       np.float32(a_neg) * L_neg[b, sl] + np.float32(r0neg)],
                      axis=1).astype(np.float32)
        rr = np.stack([np.float32(a_pos) * R_pos[b],
                       np.float32(a_neg) * R_neg[b]], axis=0).astype(np.float32)
        in_maps.append({
            "adj_rows": np.ascontiguousarray(adj[b, sl, :]),
            "h_b": np.ascontiguousarray(h[b]),
            "w_mat": W,
            "lr_rows": lr,
            "rrow": rr,
            "gamma_beta": gb,
        })
    return consts, in_maps


def _get_nc(consts):
    key = tuple(sorted((k, tuple(v) if isinstance(v, list) else v)
                       for k, v in consts.items()))
    if key not in _cache:
        _patch_tile_drain()
        _patch_bir_wait_legalize()
        _cache[key] = _build_nc(consts)
    return _cache[key]


def kernel(**inputs):
    consts, in_maps = _prep(inputs)
    nc = _get_nc(consts)
    res = run_bass_kernel_spmd(nc, in_maps, list(range(8))).results
    out = np.empty((B, N, OUT_F), np.float32)
    attention = np.empty((B, N, N), np.float32)
    for c in range(8):
        b, half = c // 2, c % 2
        sl = slice(half * ROWS, (half + 1) * ROWS)
        attention[b, sl, :] = res[c]["attn_rows"]
        out[b, sl, :] = res[c]["out_rows"]
    return out, attention
